# revision 5
# baseline (speedup 1.0000x reference)
"""Trainium2 Bass kernel for batched tiny-projection attention.

Reference computation (per batch b):
    qp = relu(q @ W1.T + b1)            [Nq, 3]
    kp = relu(k @ W2.T + b2)            [Nf, 3]
    scores = (qp @ kp.T) / sqrt(3)      [Nq, Nf]
    attn = softmax(scores, axis=-1)
    out = attn @ v                      [Nq, C]

Shapes: B=4, Nq=2048, Nf=16384, D=3, C=768, fp32.

Sharding: 8 cores = (4 batches) x (2 halves of Nq). Each core handles
q[b, h*1024:(h+1)*1024], full k[b]/v[b], so softmax is local to a core
(no cross-core reduction needed).

Device algorithm (per core), oriented for the tensor engine:
  - scores are computed TRANSPOSED: sT[m, n] = kp[m]. qp[n], because the
    attn @ v matmul needs the contraction dim (m) on partitions.
  - k-side projection emits fp16 kp DIRECTLY from the activation (one
    rounding, ~2^-11 relative) at partition blocks {0-2, 32-34}; W2 is
    fed as an exact fp16 hi+lo pair against duplicated k rows so only
    the input cast and output round lose bits.
  - q-side projection stays fp32-exact and is then split hi/lo into
    fp16 at blocks {0-2 hi, 32-34 lo}: one K=64 matmul contracts
    k16.(qhi+qlo) = k16.qp exactly.
  - exp(scale*s - shift) runs on the scalar engine straight out of PSUM,
    emitting bf16 tiles (bf16 range avoids underflow for rows whose max
    score is far below the global shift; scores >= 0 since qp,kp >= 0).
  - attn @ v accumulates in PSUM over a group of m-tiles, then is
    flushed (added) into an SBUF fp32 accumulator; v carries an extra
    ones column so the softmax denominator falls out of the same matmul.
  - The last group is small (4 m-tiles) with the per-chunk normalize +
    output DMA fused into its loop, so the 3 MB store overlaps compute.
"""

import sys

sys.path.insert(0, "/opt/trn_rl_repo")

import numpy as np

import concourse.bass as bass
import concourse.bacc as bacc
import concourse.tile as tile
from concourse import mybir
from concourse.bass_utils import run_bass_kernel_spmd

F32 = mybir.dt.float32
F16 = mybir.dt.float16
BF16 = mybir.dt.bfloat16

B, NQ_FULL, NF, D, C = 4, 2048, 16384, 3, 768
SCALE = 1.0 / np.sqrt(3.0)
NQ = NQ_FULL // 2          # per-core query rows
CA, CB = 512, C + 1 - 512  # c-chunk split of [v | ones] (769 = 512 + 257)


def build_nc(nq=NQ, nf=NF, g=16, num_devices=8):
    """Build the single-core SPMD program. g = m-tiles (of 128) per group."""
    assert nq % 512 == 0 and nf % 128 == 0
    m_tiles = nf // 128
    nchunks = nq // 128
    gm = g * 128            # field rows per group (max)
    caug = C + 1

    nc = bacc.Bacc("TRN2", target_bir_lowering=False, debug=False,
                   num_devices=num_devices)

    # wpack: cols 0-63 = wq lhsT [9, 64], cols 64-127 = wk lhsT [6, 64]
    wpack = nc.dram_tensor("wpack", [9, 128], F16, kind="ExternalInput")
    # bpack: col 0 = bq rep, col 1 = bk rep, col 2 = exp shift
    bpack = nc.dram_tensor("bpack", [128, 3], F32, kind="ExternalInput")
    qT9 = nc.dram_tensor("qT9", [9, nq], F16, kind="ExternalInput")
    kT6 = nc.dram_tensor("kT6", [6, nf], F16, kind="ExternalInput")
    vaug = nc.dram_tensor("vaug", [nf, caug], BF16, kind="ExternalInput")
    out = nc.dram_tensor("out", [nq, C], F32, kind="ExternalOutput")

    with tile.TileContext(nc) as tc, \
         tc.tile_pool(name="const", bufs=1) as const, \
         tc.tile_pool(name="kio", bufs=3) as kio, \
         tc.tile_pool(name="k16p", bufs=3) as k16p, \
         tc.tile_pool(name="vp", bufs=2 * g) as vp, \
         tc.tile_pool(name="expp", bufs=2 * g) as expp, \
         tc.tile_pool(name="outp", bufs=3) as outp, \
         tc.tile_pool(name="recp", bufs=3) as recp, \
         tc.tile_pool(name="sc_ps", bufs=3, space="PSUM") as sc_ps, \
         tc.tile_pool(name="oA_ps", bufs=3, space="PSUM") as oA_ps, \
         tc.tile_pool(name="oB_ps", bufs=2, space="PSUM") as oB_ps:

        # ---- prologue: head DMAs in dependency-critical order ----
        wpack_sb = const.tile([9, 128], F16)
        nc.sync.dma_start(wpack_sb[:], wpack[:])
        qT9_sb = const.tile([9, nq], F16)
        nc.sync.dma_start(qT9_sb[:], qT9[:])
        bpack_sb = const.tile([128, 3], F32)
        nc.sync.dma_start(bpack_sb[:], bpack[:])

        wq_sb = wpack_sb[:, 0:64]
        wk_sb = wpack_sb[0:6, 64:128]
        bq_sb = bpack_sb[0:64, 0:1]
        bk_sb = bpack_sb[0:64, 1:2]
        shift_sb = bpack_sb[:, 2:3]

        acc = const.tile([128, nchunks, caug], F32)

        # ---- q projection: exact fp32, then hi/lo fp16 split ----
        # blocks: rows 0-2 hold qp_hi, rows 32-34 hold qp_lo
        p32q = const.tile([64, nq], F32)
        qsplit = const.tile([64, nq], F16)
        qhsc = const.tile([64, nq], F16)

        def emit_projq():
            for h0 in range(0, nq, 512):
                pj = sc_ps.tile([64, 512], F32, tag="sp")
                nc.tensor.matmul(pj[:], wq_sb[:], qT9_sb[:, h0:h0 + 512],
                                 start=True, stop=True)
                nc.scalar.activation(p32q[:, h0:h0 + 512], pj[:],
                                     mybir.ActivationFunctionType.Relu,
                                     bias=bq_sb[:], scale=1.0)
                nc.vector.tensor_copy(qsplit[:, h0:h0 + 512],
                                      p32q[:, h0:h0 + 512])
                nc.scalar.copy(qhsc[32:64, h0:h0 + 512],
                               p32q[32:64, h0:h0 + 512])
                nc.vector.tensor_sub(qsplit[32:64, h0:h0 + 512],
                                     p32q[32:64, h0:h0 + 512],
                                     qhsc[32:64, h0:h0 + 512])

        # ---- k projection: fp16 out straight from the activation ----
        # k16 tile rows {0-2, 32-34} hold kp (two copies); zeros elsewhere
        def emit_projk(m0_tiles, size):
            n = size * 128
            kt = kio.tile([6, gm], F16)
            c0 = m0_tiles * 128
            nc.sync.dma_start(kt[:, 0:n], kT6[:, c0:c0 + n])
            k16 = k16p.tile([64, gm], F16)
            for h0 in range(0, n, 512):
                pj = sc_ps.tile([64, 512], F32, tag="sp")
                nc.tensor.matmul(pj[:], wk_sb[:], kt[:, h0:h0 + 512],
                                 start=True, stop=True)
                nc.scalar.activation(k16[:, h0:h0 + 512], pj[:],
                                     mybir.ActivationFunctionType.Relu,
                                     bias=bk_sb[:], scale=1.0)
            return k16

        def emit_v(m0_tiles, size):
            vts = []
            for t in range(size):
                m0 = (m0_tiles + t) * 128
                vt = vp.tile([128, caug], BF16)
                nc.sync.dma_start(vt[:], vaug[m0:m0 + 128, :])
                vts.append(vt)
            return vts

        def emit_scores(k16, ts, h_major=False):
            """scores + exp for m-tiles ts (local idx within group).
            h_major orders the low n-columns of every tile first, so the
            first attn chunk's dependencies complete earliest."""
            es = []
            for t in ts:
                et = expp.tile([128, nq], BF16)
                es.append(et)
            ts = list(ts)
            order = [(h, j) for h in range(nq // 512) for j in range(len(ts))]
            if not h_major:
                order = [(h, j) for j in range(len(ts)) for h in range(nq // 512)]
            for h, j in order:
                t = ts[j]
                sp = sc_ps.tile([128, 512], F32, tag="sp")
                nc.tensor.matmul(sp[:], k16[:, t * 128:(t + 1) * 128],
                                 qsplit[:, h * 512:(h + 1) * 512],
                                 start=True, stop=True)
                nc.scalar.activation(es[j][:, h * 512:(h + 1) * 512], sp[:],
                                     mybir.ActivationFunctionType.Exp,
                                     bias=shift_sb[:], scale=float(SCALE))
            return es

        def emit_attn_chunk(first_group, ci, es, vts):
            n = len(es)
            pA = oA_ps.tile([128, CA], F32)
            pB = oB_ps.tile([128, CB], F32)
            for i in range(n):
                e = es[i][:, ci * 128:(ci + 1) * 128]
                nc.tensor.matmul(pA[:], e, vts[i][:, 0:CA],
                                 start=(i == 0), stop=(i == n - 1))
                nc.tensor.matmul(pB[:], e, vts[i][:, CA:caug],
                                 start=(i == 0), stop=(i == n - 1))
            if first_group:
                nc.vector.tensor_copy(acc[:, ci, 0:CA], pA[:])
                nc.vector.tensor_copy(acc[:, ci, CA:caug], pB[:])
            else:
                nc.vector.tensor_add(acc[:, ci, 0:CA], acc[:, ci, 0:CA], pA[:])
                nc.vector.tensor_add(acc[:, ci, CA:caug], acc[:, ci, CA:caug],
                                     pB[:])

        def emit_finale(ci):
            rec = recp.tile([128, 1], F32)
            nc.vector.reciprocal(rec[:], acc[:, ci, C:caug])
            ot = outp.tile([128, C], F32)
            nc.vector.tensor_scalar_mul(ot[:], acc[:, ci, 0:C], rec[:])
            nc.sync.dma_start(out[ci * 128:(ci + 1) * 128, :], ot[:])

        # ---- software-pipelined main loop ----
        # small groups first (attn starts waiting on only a few exp
        # tiles); small last group so the finale/output DMA overlaps.
        if m_tiles == 128 and g == 16:
            sizes = [4, 4, 8] + [16] * 6 + [12, 4]
        else:
            ngroups = m_tiles // g
            assert g * ngroups == m_tiles
            sizes = [g] * ngroups
        starts = [sum(sizes[:i]) for i in range(len(sizes))]
        n_g = len(sizes)

        # prefetch: k-projection runs two groups ahead so its relu sits
        # ahead of the exp burst in the scalar engine's in-order queue
        emit_projq()
        ks = {0: emit_projk(starts[0], sizes[0])}
        if n_g > 1:
            ks[1] = emit_projk(starts[1], sizes[1])
        v_cur = emit_v(starts[0], sizes[0])
        e_cur = emit_scores(ks[0], range(sizes[0]), h_major=True)

        for gi in range(n_g):
            last = gi + 1 >= n_g
            if gi + 2 < n_g:
                ks[gi + 2] = emit_projk(starts[gi + 2], sizes[gi + 2])
            if not last:
                v_nxt = emit_v(starts[gi + 1], sizes[gi + 1])
                e_nxt = []
            # distribute next group's score matmuls across this group's
            # attn chunks to keep PE dense and ACT fed early
            for ci in range(nchunks):
                emit_attn_chunk(gi == 0, ci, e_cur, v_cur)
                if last:
                    emit_finale(ci)
                else:
                    nnx = sizes[gi + 1]
                    per = (nnx + nchunks - 1) // nchunks
                    ts = range(ci * per, min((ci + 1) * per, nnx))
                    e_nxt.extend(emit_scores(ks[gi + 1], ts))
            if not last:
                v_cur, e_cur = v_nxt, e_nxt

    nc.finalize()
    return nc


def _split16(x):
    hi = x.astype(np.float16)
    lo = (x - hi.astype(np.float32)).astype(np.float16)
    return hi, lo


def _wlhs_q(W):
    """q lhsT [9, 64]: K rows = [Whi, Whi, Wlo] pairing qT9 rows
    [qhi, qlo, qhi]; projected row e lands at partitions {e, 32+e}."""
    Whi, Wlo = _split16(W.astype(np.float32))
    m = np.zeros((9, 64), np.float16)
    for e in range(3):
        for d in range(3):
            for base in (0, 32):
                m[0 + d, base + e] = Whi[e, d]
                m[3 + d, base + e] = Whi[e, d]
                m[6 + d, base + e] = Wlo[e, d]
    return m


def _wlhs_k(W):
    """k lhsT [6, 64]: K rows = [Whi, Wlo] pairing kT6 rows [k16, k16];
    projected row e lands at partitions {e, 32+e} (two copies)."""
    Whi, Wlo = _split16(W.astype(np.float32))
    m = np.zeros((6, 64), np.float16)
    for e in range(3):
        for d in range(3):
            for base in (0, 32):
                m[0 + d, base + e] = Whi[e, d]
                m[3 + d, base + e] = Wlo[e, d]
    return m


def _brep(b):
    """bias [128, 1]: b[e] at partitions {e, 32+e}, zero elsewhere."""
    m = np.zeros((128, 1), np.float32)
    for e in range(3):
        for base in (0, 32):
            m[base + e, 0] = b[e]
    return m


def _t9(x2d):
    """[N, 3] -> [9, N] fp16 rows [hi, lo, hi]."""
    xT = np.ascontiguousarray(x2d.T.astype(np.float32))
    hi, lo = _split16(xT)
    return np.concatenate([hi, lo, hi], axis=0)


def _t6(x2d):
    """[N, 3] -> [6, N] fp16: the fp16 cast duplicated (hi+lo W rows)."""
    xT = np.ascontiguousarray(x2d.T.astype(np.float32)).astype(np.float16)
    return np.concatenate([xT, xT], axis=0)


def _host_prep(q, k, v, W1, b1, W2, b2):
    """Build per-core input maps (layout/dtype prep only)."""
    import ml_dtypes
    wpack = np.zeros((9, 128), np.float16)
    wpack[:, 0:64] = _wlhs_q(W1)
    wpack[0:6, 64:128] = _wlhs_k(W2)

    in_maps = []
    per_batch = {}
    for b in range(B):
        # cheap per-batch upper bound on max score -> exp(s - shift) <= 1
        qp = np.maximum(q[b].astype(np.float32) @ W1.T.astype(np.float32)
                        + b1.astype(np.float32), 0.0)
        kp = np.maximum(k[b].astype(np.float32) @ W2.T.astype(np.float32)
                        + b2.astype(np.float32), 0.0)
        bound = SCALE * float(qp.max(axis=0) @ kp.max(axis=0))
        va = np.ones((NF, C + 1), np.float32)
        va[:, :C] = v[b]
        bpack = np.zeros((128, 3), np.float32)
        bpack[:, 0:1] = _brep(b1)
        bpack[:, 1:2] = _brep(b2)
        bpack[:, 2] = -bound
        per_batch[b] = {
            "kT6": _t6(k[b]),
            "vaug": va.astype(ml_dtypes.bfloat16),
            "bpack": bpack,
        }
    for core in range(8):
        b, h = core // 2, core % 2
        qs = q[b, h * NQ:(h + 1) * NQ, :]
        in_maps.append({
            "qT9": _t9(qs),
            "wpack": wpack,
            **per_batch[b],
        })
    return in_maps


_NC_CACHE = {}


def kernel(q, k, v, W1, b1, W2, b2, _trace=False):
    q, k, v = np.asarray(q), np.asarray(k), np.asarray(v)
    W1, b1 = np.asarray(W1), np.asarray(b1)
    W2, b2 = np.asarray(W2), np.asarray(b2)

    if "nc" not in _NC_CACHE:
        _NC_CACHE["nc"] = build_nc()
    nc = _NC_CACHE["nc"]

    in_maps = _host_prep(q, k, v, W1, b1, W2, b2)
    res = run_bass_kernel_spmd(nc, in_maps, list(range(8)), trace=_trace)

    out = np.empty((B, NQ_FULL, C), np.float32)
    for core in range(8):
        b, h = core // 2, core % 2
        out[b, h * NQ:(h + 1) * NQ, :] = res.results[core]["out"]
    if _trace:
        return out, res
    return out
